# revision 8
# baseline (speedup 1.0000x reference)
"""AtomToPair GNN message-passing kernel for 8 TRN2 NeuronCores.

Math (per molecule, A=64 atoms, F=C=128):
    h0[i,j] = MLP([x_i, x_j]),  h1[i,j] = MLP([x_j, x_i]) = h0[j,i]
    out[i,j] = h0[i,j] + h0[j,i]
so a single MLP pass over all A*A ordered pairs suffices; the final
transposed add (out = H + H^T per molecule) runs on the HOST during
unsharding — on-chip it would need strided mirror reads that measure
~3.4 cyc/elem on the DVE, three times the cost of the linear drains.

Layer 1 factors per atom: [x_i,x_j]@W0 = x_i@W0top + x_j@W0bot, computed
on the TensorEngine as accumulated bf16 matmuls whose moving operand
reads xT with broadcast access patterns (no pair tensor materialized).

Per-core pipeline (4 molecules x 4 groups of 16 i-rows = 1024 pairs):
  PE   : L1 = 4 matmuls N=512 (w0t,w0t,w0b,w0b) -> psY [C,1024]
         L2 = 2 matmuls N=512 (w1)              -> psH [C,1024]
  ACT  : relu1 (+b0) psY -> y1 bf16   (one FD=1024 op per group)
  DVE  : relu2 (+b1) psH -> hg bf16   (one FD=1024 op per group,
                                       a few groups on ACT to balance)
  DMA  : ship each group's H rows to HBM as they complete
L2 of group g is emitted one group behind L1 (skew) so each PSUM
drain has a full-slot window: psY bufs=2 + psH bufs=2 = all 8 banks.

Output is the full H grid in bf16 ([C, 4096] per molecule); the host
computes out[b] = H + H^T and upcasts to fp32. Weights/x are bf16
on-chip; PSUM accumulation stays fp32.

Sharding: data-parallel over batch — each of the 8 cores handles
B/8 = 4 molecules with fully replicated weights.
"""

import sys

sys.path.insert(0, "/opt/trn_rl_repo")

import os

import numpy as np

B, A, F, C = 32, 64, 128, 128
NCORES = 8
MPC = B // NCORES          # molecules per core
PAIRS = A * A              # 4096
IB = 8                     # i-block (rows per chunk)
NCHUNK = A // IB           # 8 chunks per molecule
NG = 4                     # groups (of 2 chunks / 16 rows) per molecule
GW = 2 * IB * A            # pair-columns per group (1024)

# packed bf16 param columns: xT | w0t | w0b | w1
XB_OFF = 0
W0T_OFF = MPC * A
W0B_OFF = MPC * A + C
W1_OFF = MPC * A + 2 * C
PB_COLS = MPC * A + 3 * C

N_WARMUP = int(os.environ.get("ATOMPAIR_KWARM", "4"))
# global group indices (0..15) whose relu2 runs on ACT instead of DVE
_R2A = os.environ.get("ATOMPAIR_KR2ACT", "")
RELU2_ACT = set(int(s) for s in _R2A.split(",") if s != "")

_compiled = {}


def _build(fused=False):
    import concourse.bass as bass
    import concourse.tile as tile
    from concourse import bacc, mybir

    fp32 = mybir.dt.float32
    bf16 = mybir.dt.bfloat16
    nc = bacc.Bacc("TRN2", target_bir_lowering=False, debug=False,
                   num_devices=NCORES)

    pb16 = nc.dram_tensor("pb16", [128, PB_COLS], bf16,
                          kind="ExternalInput").ap()
    pf32 = nc.dram_tensor("pf32", [128, 2], fp32, kind="ExternalInput").ap()
    out = nc.dram_tensor("out", [C, MPC * PAIRS], bf16,
                         kind="ExternalOutput").ap()

    Relu = mybir.ActivationFunctionType.Relu
    add_op = mybir.AluOpType.add
    max_op = mybir.AluOpType.max

    with tile.TileContext(nc) as tc:
        with (
            tc.tile_pool(name="const", bufs=1) as const_pool,
            tc.tile_pool(name="warm", bufs=1) as warm_pool,
            tc.tile_pool(name="y1", bufs=3) as y1_pool,
            tc.tile_pool(name="hg", bufs=3) as hg_pool,
            tc.tile_pool(name="psY", bufs=2, space="PSUM") as psY_pool,
            tc.tile_pool(name="psH", bufs=2, space="PSUM") as psH_pool,
        ):
            pb = const_pool.tile([128, PB_COLS], bf16, tag="pb")
            pf = const_pool.tile([128, 2], fp32, tag="pf")
            # spread input-DMA issue (each costs ~0.6us of issuing-engine
            # time) across otherwise-idle queues; weights first so the
            # first L1 can start ASAP
            nc.sync.dma_start(pb[:, W0T_OFF:], pb16[:, W0T_OFF:])
            nc.sync.dma_start(pf[:], pf32[:])
            dma_engs = [nc.scalar, nc.gpsimd, nc.scalar, nc.sync]
            for m in range(MPC):
                dma_engs[m].dma_start(pb[:, m * A: (m + 1) * A],
                                      pb16[:, m * A: (m + 1) * A])

            # PE warm-up: dummy matmuls with no input dependency keep the
            # HAM activity window busy during the input DMA so real
            # matmuls start at the full 2.4 GHz clock.
            if N_WARMUP > 0:
                wsrc = warm_pool.tile([128, 512], bf16, tag="wsrc")
                nc.gpsimd.memset(wsrc[:], 0.0)
                for w in range(N_WARMUP):
                    wp = psH_pool.tile([C, GW], fp32, tag="psh")
                    nc.tensor.matmul(wp[:, :512], wsrc[:, :128], wsrc[:],
                                     start=True, stop=True)

            w0t_s = pb[:, W0T_OFF: W0T_OFF + C]
            w0b_s = pb[:, W0B_OFF: W0B_OFF + C]
            w1_s = pb[:, W1_OFF: W1_OFF + C]
            b0_s = pf[:, 0:1]
            b1_s = pf[:, 1:2]

            units = [(m, q) for m in range(MPC) for q in range(NG)]
            state = {}

            def emit_L1(idx):
                m, q = units[idx]
                xm = pb[:, XB_OFF + m * A: XB_OFF + (m + 1) * A]
                psy = psY_pool.tile([C, GW], fp32, tag="psy")
                # moving free dim caps at 512 -> per-chunk matmuls, with
                # same-weight matmuls adjacent so LDWEIGHTS can overlap
                views = []
                for h in (0, 1):
                    k = 2 * q + h
                    xi = xm[:, k * IB: (k + 1) * IB]
                    rhs_i = xi.unsqueeze(2).to_broadcast((F, IB, A))
                    ps3 = psy[:, h * IB * A: (h + 1) * IB * A].rearrange(
                        "c (i j) -> c i j", i=IB)
                    views.append((ps3, rhs_i))
                rhs_j = xm.unsqueeze(1).to_broadcast((F, IB, A))
                for ps3, rhs_i in views:
                    nc.tensor.matmul(ps3, w0t_s, rhs_i,
                                     start=True, stop=False)
                for ps3, _ in views:
                    nc.tensor.matmul(ps3, w0b_s, rhs_j,
                                     start=False, stop=True)
                # relu1 queued on ACT immediately; runs as soon as L1 lands
                y1t = y1_pool.tile([C, GW], bf16, tag="y1t")
                nc.scalar.activation(y1t[:], psy[:], Relu, bias=b0_s)
                state[idx] = y1t

            def emit_L2(idx):
                m, q = units[idx]
                y1t = state.pop(idx)
                psh = psH_pool.tile([C, GW], fp32, tag="psh")
                for h in (0, 1):
                    nc.tensor.matmul(psh[:, h * IB * A: (h + 1) * IB * A],
                                     w1_s,
                                     y1t[:, h * IB * A: (h + 1) * IB * A],
                                     start=True, stop=True)
                hg = hg_pool.tile([C, GW], bf16, tag="hg")
                if m * NG + q in RELU2_ACT:
                    nc.scalar.activation(hg[:], psh[:], Relu, bias=b1_s)
                else:
                    nc.vector.tensor_scalar(hg[:], psh[:], b1_s, 0.0,
                                            add_op, max_op)
                lo = m * PAIRS + q * GW
                nc.gpsimd.dma_start(out[:, lo: lo + GW], hg[:])

            # software-pipelined emission, skew-1 between L1 and L2
            for idx in range(len(units) + 1):
                if idx < len(units):
                    emit_L1(idx)
                if idx >= 1:
                    emit_L2(idx - 1)
    nc.compile()
    return nc


def _get_compiled(fused=False):
    if fused not in _compiled:
        _compiled[fused] = _build(fused)
    return _compiled[fused]


def _shard_inputs(x, W0, b0, W1, b1):
    import ml_dtypes

    bf = ml_dtypes.bfloat16
    pf32 = np.stack([b0, b1], axis=1).astype(np.float32)  # [128, 2]
    w_cols = np.concatenate([W0[:F], W0[F:], W1], axis=1).astype(bf)
    in_maps = []
    for c in range(NCORES):
        xs = x[c * MPC: (c + 1) * MPC]                    # [MPC, A, F]
        xTs = xs.transpose(2, 0, 1).reshape(F, MPC * A)
        pb16 = np.ascontiguousarray(
            np.concatenate([xTs.astype(bf), w_cols], axis=1))
        in_maps.append({"pb16": pb16, "pf32": pf32})
    return in_maps


def _unshard(results):
    """[C, MPC*PAIRS] bf16 per core -> full (B, A*A, C) fp32 = H + H^T."""
    full = np.empty((B, A * A, C), dtype=np.float32)
    for c in range(NCORES):
        o = np.asarray(results[c]["out"], dtype=np.float32)
        for m in range(MPC):
            bidx = c * MPC + m
            h = o[:, m * PAIRS: (m + 1) * PAIRS].reshape(C, A, A)
            hsum = h + h.transpose(0, 2, 1)        # H[i,j] + H[j,i]
            full[bidx] = hsum.reshape(C, PAIRS).T
    return full


def kernel(x, W0, b0, W1, b1):
    from concourse.bass_utils import run_bass_kernel_spmd

    x = np.asarray(x, dtype=np.float32)
    W0 = np.asarray(W0, dtype=np.float32)
    b0 = np.asarray(b0, dtype=np.float32)
    W1 = np.asarray(W1, dtype=np.float32)
    b1 = np.asarray(b1, dtype=np.float32)

    in_maps = _shard_inputs(x, W0, b0, W1, b1)
    nc = _get_compiled(fused=False)
    res = run_bass_kernel_spmd(nc, in_maps, core_ids=list(range(NCORES)))
    return _unshard(res.results)


# revision 9
# speedup vs baseline: 1.0229x; 1.0229x over previous
"""AtomToPair GNN message-passing kernel for 8 TRN2 NeuronCores.

Math (per molecule, A=64 atoms, F=C=128):
    h0[i,j] = MLP([x_i, x_j]),  h1[i,j] = MLP([x_j, x_i]) = h0[j,i]
    out[i,j] = h0[i,j] + h0[j,i]
so a single MLP pass over all A*A ordered pairs suffices; the final
transposed add (out = H + H^T per molecule) runs on the HOST during
unsharding — on-chip it would need strided mirror reads that measure
~3.4 cyc/elem on the DVE, three times the cost of the linear drains.

Layer 1 factors per atom: [x_i,x_j]@W0 = x_i@W0top + x_j@W0bot, computed
on the TensorEngine as accumulated bf16 matmuls whose moving operand
reads xT with broadcast access patterns (no pair tensor materialized).

Per-core pipeline (4 molecules x 4 groups of 16 i-rows = 1024 pairs):
  PE   : L1 = 4 matmuls N=512 (w0t,w0t,w0b,w0b) -> psY [C,1024]
         L2 = 2 matmuls N=512 (w1)              -> psH [C,1024]
  ACT  : relu1 (+b0) psY -> y1 bf16   (one FD=1024 op per group)
  DVE  : relu2 (+b1) psH -> hg bf16   (one FD=1024 op per group,
                                       a few groups on ACT to balance)
  DMA  : ship each group's H rows to HBM as they complete
L2 of group g is emitted one group behind L1 (skew) so each PSUM
drain has a full-slot window: psY bufs=2 + psH bufs=2 = all 8 banks.

Output is the full H grid in bf16 ([C, 4096] per molecule); the host
computes out[b] = H + H^T and upcasts to fp32. Weights/x are bf16
on-chip; PSUM accumulation stays fp32.

Sharding: data-parallel over batch — each of the 8 cores handles
B/8 = 4 molecules with fully replicated weights.
"""

import sys

sys.path.insert(0, "/opt/trn_rl_repo")

import os

import numpy as np

B, A, F, C = 32, 64, 128, 128
NCORES = 8
MPC = B // NCORES          # molecules per core
PAIRS = A * A              # 4096
IB = 8                     # i-block (rows per chunk)
NCHUNK = A // IB           # 8 chunks per molecule
NG = 4                     # groups (of 2 chunks / 16 rows) per molecule
GW = 2 * IB * A            # pair-columns per group (1024)

# packed bf16 param columns: xT | w0t | w0b | w1
XB_OFF = 0
W0T_OFF = MPC * A
W0B_OFF = MPC * A + C
W1_OFF = MPC * A + 2 * C
PB_COLS = MPC * A + 3 * C

N_WARMUP = int(os.environ.get("ATOMPAIR_KWARM", "4"))
# global group indices (0..15) whose relu2 runs on ACT instead of DVE
_R2A = os.environ.get("ATOMPAIR_KR2ACT", "")
RELU2_ACT = set(int(s) for s in _R2A.split(",") if s != "")

_compiled = {}


def _build(fused=False):
    import concourse.bass as bass
    import concourse.tile as tile
    from concourse import bacc, mybir

    fp32 = mybir.dt.float32
    bf16 = mybir.dt.bfloat16
    nc = bacc.Bacc("TRN2", target_bir_lowering=False, debug=False,
                   num_devices=NCORES)

    pb16 = nc.dram_tensor("pb16", [128, PB_COLS], bf16,
                          kind="ExternalInput").ap()
    pf32 = nc.dram_tensor("pf32", [128, 2], fp32, kind="ExternalInput").ap()
    out = nc.dram_tensor("out", [C, MPC * PAIRS], bf16,
                         kind="ExternalOutput").ap()

    Relu = mybir.ActivationFunctionType.Relu
    add_op = mybir.AluOpType.add
    max_op = mybir.AluOpType.max

    with tile.TileContext(nc) as tc:
        with (
            tc.tile_pool(name="const", bufs=1) as const_pool,
            tc.tile_pool(name="warm", bufs=1) as warm_pool,
            tc.tile_pool(name="y1", bufs=3) as y1_pool,
            tc.tile_pool(name="hg", bufs=3) as hg_pool,
            tc.tile_pool(name="psY", bufs=2, space="PSUM") as psY_pool,
            tc.tile_pool(name="psH", bufs=2, space="PSUM") as psH_pool,
        ):
            pb = const_pool.tile([128, PB_COLS], bf16, tag="pb")
            pf = const_pool.tile([128, 2], fp32, tag="pf")
            # spread input-DMA issue (each costs ~0.6us of issuing-engine
            # time) across otherwise-idle queues; weights first so the
            # first L1 can start ASAP
            nc.sync.dma_start(pb[:, W0T_OFF:], pb16[:, W0T_OFF:])
            nc.sync.dma_start(pf[:], pf32[:])
            dma_engs = [nc.scalar, nc.gpsimd, nc.scalar, nc.sync]
            for m in range(MPC):
                dma_engs[m].dma_start(pb[:, m * A: (m + 1) * A],
                                      pb16[:, m * A: (m + 1) * A])

            # PE warm-up: dummy matmuls with no input dependency keep the
            # HAM activity window busy during the input DMA so real
            # matmuls start at the full 2.4 GHz clock.
            if N_WARMUP > 0:
                wsrc = warm_pool.tile([128, 512], bf16, tag="wsrc")
                nc.gpsimd.memset(wsrc[:], 0.0)
                for w in range(N_WARMUP):
                    wp = psH_pool.tile([C, GW], fp32, tag="psh")
                    nc.tensor.matmul(wp[:, :512], wsrc[:, :128], wsrc[:],
                                     start=True, stop=True)

            w0t_s = pb[:, W0T_OFF: W0T_OFF + C]
            w0b_s = pb[:, W0B_OFF: W0B_OFF + C]
            w1_s = pb[:, W1_OFF: W1_OFF + C]
            b0_s = pf[:, 0:1]
            b1_s = pf[:, 1:2]

            units = [(m, q) for m in range(MPC) for q in range(NG)]
            state = {}

            def emit_L1(idx):
                m, q = units[idx]
                xm = pb[:, XB_OFF + m * A: XB_OFF + (m + 1) * A]
                psy = psY_pool.tile([C, GW], fp32, tag="psy")
                # moving free dim caps at 512 -> per-chunk matmuls, with
                # same-weight matmuls adjacent so LDWEIGHTS can overlap
                views = []
                for h in (0, 1):
                    k = 2 * q + h
                    xi = xm[:, k * IB: (k + 1) * IB]
                    rhs_i = xi.unsqueeze(2).to_broadcast((F, IB, A))
                    ps3 = psy[:, h * IB * A: (h + 1) * IB * A].rearrange(
                        "c (i j) -> c i j", i=IB)
                    views.append((ps3, rhs_i))
                rhs_j = xm.unsqueeze(1).to_broadcast((F, IB, A))
                for ps3, rhs_i in views:
                    nc.tensor.matmul(ps3, w0t_s, rhs_i,
                                     start=True, stop=False)
                for ps3, _ in views:
                    nc.tensor.matmul(ps3, w0b_s, rhs_j,
                                     start=False, stop=True)
                # relu1 queued on ACT immediately; runs as soon as L1 lands
                y1t = y1_pool.tile([C, GW], bf16, tag="y1t")
                nc.scalar.activation(y1t[:], psy[:], Relu, bias=b0_s)
                state[idx] = y1t

            def emit_L2(idx):
                m, q = units[idx]
                y1t = state.pop(idx)
                psh = psH_pool.tile([C, GW], fp32, tag="psh")
                for h in (0, 1):
                    nc.tensor.matmul(psh[:, h * IB * A: (h + 1) * IB * A],
                                     w1_s,
                                     y1t[:, h * IB * A: (h + 1) * IB * A],
                                     start=True, stop=True)
                hg = hg_pool.tile([C, GW], bf16, tag="hg")
                if m * NG + q in RELU2_ACT:
                    nc.scalar.activation(hg[:], psh[:], Relu, bias=b1_s)
                else:
                    nc.vector.tensor_scalar(hg[:], psh[:], b1_s, 0.0,
                                            add_op, max_op)
                lo = m * PAIRS + q * GW
                nc.sync.dma_start(out[:, lo: lo + GW], hg[:])

            # software-pipelined emission, skew-1 between L1 and L2
            for idx in range(len(units) + 1):
                if idx < len(units):
                    emit_L1(idx)
                if idx >= 1:
                    emit_L2(idx - 1)
    nc.compile()
    return nc


def _get_compiled(fused=False):
    if fused not in _compiled:
        _compiled[fused] = _build(fused)
    return _compiled[fused]


def _shard_inputs(x, W0, b0, W1, b1):
    import ml_dtypes

    bf = ml_dtypes.bfloat16
    pf32 = np.stack([b0, b1], axis=1).astype(np.float32)  # [128, 2]
    w_cols = np.concatenate([W0[:F], W0[F:], W1], axis=1).astype(bf)
    in_maps = []
    for c in range(NCORES):
        xs = x[c * MPC: (c + 1) * MPC]                    # [MPC, A, F]
        xTs = xs.transpose(2, 0, 1).reshape(F, MPC * A)
        pb16 = np.ascontiguousarray(
            np.concatenate([xTs.astype(bf), w_cols], axis=1))
        in_maps.append({"pb16": pb16, "pf32": pf32})
    return in_maps


def _unshard(results):
    """[C, MPC*PAIRS] bf16 per core -> full (B, A*A, C) fp32 = H + H^T."""
    full = np.empty((B, A * A, C), dtype=np.float32)
    for c in range(NCORES):
        o = np.asarray(results[c]["out"], dtype=np.float32)
        for m in range(MPC):
            bidx = c * MPC + m
            h = o[:, m * PAIRS: (m + 1) * PAIRS].reshape(C, A, A)
            hsum = h + h.transpose(0, 2, 1)        # H[i,j] + H[j,i]
            full[bidx] = hsum.reshape(C, PAIRS).T
    return full


def kernel(x, W0, b0, W1, b1):
    from concourse.bass_utils import run_bass_kernel_spmd

    x = np.asarray(x, dtype=np.float32)
    W0 = np.asarray(W0, dtype=np.float32)
    b0 = np.asarray(b0, dtype=np.float32)
    W1 = np.asarray(W1, dtype=np.float32)
    b1 = np.asarray(b1, dtype=np.float32)

    in_maps = _shard_inputs(x, W0, b0, W1, b1)
    nc = _get_compiled(fused=False)
    res = run_bass_kernel_spmd(nc, in_maps, core_ids=list(range(NCORES)))
    return _unshard(res.results)


# revision 10
# speedup vs baseline: 1.1786x; 1.1522x over previous
"""AtomToPair GNN message-passing kernel for 8 TRN2 NeuronCores.

Math (per molecule, A=64 atoms, F=C=128):
    h0[i,j] = MLP([x_i, x_j]),  h1[i,j] = MLP([x_j, x_i]) = h0[j,i]
    out[i,j] = h0[i,j] + h0[j,i]
so a single MLP pass over all A*A ordered pairs suffices; the final
transposed add (out = H + H^T per molecule) runs on the HOST during
unsharding — on-chip it would need strided mirror reads that measure
~3.4 cyc/elem on the DVE, three times the cost of the linear drains.

Layer 1 factors per atom: [x_i,x_j]@W0 = x_i@W0top + x_j@W0bot, computed
on the TensorEngine as accumulated bf16 matmuls whose moving operand
reads xT with broadcast access patterns (no pair tensor materialized).

Per-core pipeline (4 molecules x 4 groups of 16 i-rows = 1024 pairs):
  PE   : L1 = 4 matmuls N=512 (w0t,w0t,w0b,w0b) -> psY [C,1024]
         L2 = 2 matmuls N=512 (w1)              -> psH [C,1024]
  ACT  : relu1 (+b0) psY -> y1 bf16   (one FD=1024 op per group)
  DVE  : relu2 (+b1) psH -> hg bf16   (one FD=1024 op per group,
                                       a few groups on ACT to balance)
  DMA  : ship each group's H rows to HBM as they complete
L2 of group g is emitted one group behind L1 (skew) so each PSUM
drain has a full-slot window: psY bufs=2 + psH bufs=2 = all 8 banks.

Output is the full H grid in bf16 ([C, 4096] per molecule); the host
computes out[b] = H + H^T and upcasts to fp32. Weights/x are bf16
on-chip; PSUM accumulation stays fp32.

Sharding: data-parallel over batch — each of the 8 cores handles
B/8 = 4 molecules with fully replicated weights.
"""

import sys

sys.path.insert(0, "/opt/trn_rl_repo")

import os

import numpy as np

B, A, F, C = 32, 64, 128, 128
NCORES = 8
MPC = B // NCORES          # molecules per core
PAIRS = A * A              # 4096
IB = 8                     # i-block (rows per chunk)
NCHUNK = A // IB           # 8 chunks per molecule
NG = 4                     # groups (of 2 chunks / 16 rows) per molecule
GW = 2 * IB * A            # pair-columns per group (1024)

# packed bf16 param columns: xT | w0t | w0b | w1
XB_OFF = 0
W0T_OFF = MPC * A
W0B_OFF = MPC * A + C
W1_OFF = MPC * A + 2 * C
PB_COLS = MPC * A + 3 * C

N_WARMUP = int(os.environ.get("ATOMPAIR_KWARM", "4"))
# global group indices (0..15) whose relu2 runs on ACT instead of DVE
_R2A = os.environ.get("ATOMPAIR_KR2ACT", "")
RELU2_ACT = set(int(s) for s in _R2A.split(",") if s != "")

_compiled = {}


def _build(fused=False):
    import concourse.bass as bass
    import concourse.tile as tile
    from concourse import bacc, mybir

    fp32 = mybir.dt.float32
    bf16 = mybir.dt.bfloat16
    nc = bacc.Bacc("TRN2", target_bir_lowering=False, debug=False,
                   num_devices=NCORES)

    pb16 = nc.dram_tensor("pb16", [128, PB_COLS], bf16,
                          kind="ExternalInput").ap()
    pf32 = nc.dram_tensor("pf32", [128, 2], fp32, kind="ExternalInput").ap()
    out = nc.dram_tensor("out", [C, MPC * PAIRS], bf16,
                         kind="ExternalOutput").ap()

    Relu = mybir.ActivationFunctionType.Relu
    add_op = mybir.AluOpType.add
    max_op = mybir.AluOpType.max

    with tile.TileContext(nc) as tc:
        with (
            tc.tile_pool(name="const", bufs=1) as const_pool,
            tc.tile_pool(name="warm", bufs=1) as warm_pool,
            tc.tile_pool(name="y1", bufs=3) as y1_pool,
            tc.tile_pool(name="hg", bufs=3) as hg_pool,
            tc.tile_pool(name="psY", bufs=2, space="PSUM") as psY_pool,
            tc.tile_pool(name="psH", bufs=2, space="PSUM") as psH_pool,
        ):
            pb = const_pool.tile([128, PB_COLS], bf16, tag="pb")
            pf = const_pool.tile([128, 2], fp32, tag="pf")
            # spread input-DMA issue (each costs ~0.6us of issuing-engine
            # time) across otherwise-idle queues; weights first so the
            # first L1 can start ASAP
            nc.sync.dma_start(pb[:, W0T_OFF:], pb16[:, W0T_OFF:])
            nc.sync.dma_start(pf[:], pf32[:])
            for m in range(MPC):
                nc.sync.dma_start(pb[:, m * A: (m + 1) * A],
                                  pb16[:, m * A: (m + 1) * A])

            # PE warm-up: dummy matmuls with no input dependency keep the
            # HAM activity window busy during the input DMA so real
            # matmuls start at the full 2.4 GHz clock.
            if N_WARMUP > 0:
                wsrc = warm_pool.tile([128, 512], bf16, tag="wsrc")
                nc.gpsimd.memset(wsrc[:], 0.0)
                for w in range(N_WARMUP):
                    wp = psH_pool.tile([C, GW], fp32, tag="psh")
                    nc.tensor.matmul(wp[:, :512], wsrc[:, :128], wsrc[:],
                                     start=True, stop=True)

            w0t_s = pb[:, W0T_OFF: W0T_OFF + C]
            w0b_s = pb[:, W0B_OFF: W0B_OFF + C]
            w1_s = pb[:, W1_OFF: W1_OFF + C]
            b0_s = pf[:, 0:1]
            b1_s = pf[:, 1:2]

            units = [(m, q) for m in range(MPC) for q in range(NG)]
            state = {}

            def emit_L1(idx):
                m, q = units[idx]
                xm = pb[:, XB_OFF + m * A: XB_OFF + (m + 1) * A]
                psy = psY_pool.tile([C, GW], fp32, tag="psy")
                # moving free dim caps at 512 -> per-chunk matmuls, with
                # same-weight matmuls adjacent so LDWEIGHTS can overlap
                views = []
                for h in (0, 1):
                    k = 2 * q + h
                    xi = xm[:, k * IB: (k + 1) * IB]
                    rhs_i = xi.unsqueeze(2).to_broadcast((F, IB, A))
                    ps3 = psy[:, h * IB * A: (h + 1) * IB * A].rearrange(
                        "c (i j) -> c i j", i=IB)
                    views.append((ps3, rhs_i))
                rhs_j = xm.unsqueeze(1).to_broadcast((F, IB, A))
                for ps3, rhs_i in views:
                    nc.tensor.matmul(ps3, w0t_s, rhs_i,
                                     start=True, stop=False)
                for ps3, _ in views:
                    nc.tensor.matmul(ps3, w0b_s, rhs_j,
                                     start=False, stop=True)
                # relu1 queued on ACT immediately; runs as soon as L1 lands
                y1t = y1_pool.tile([C, GW], bf16, tag="y1t")
                nc.scalar.activation(y1t[:], psy[:], Relu, bias=b0_s)
                state[idx] = y1t

            def emit_L2(idx):
                m, q = units[idx]
                y1t = state.pop(idx)
                psh = psH_pool.tile([C, GW], fp32, tag="psh")
                for h in (0, 1):
                    nc.tensor.matmul(psh[:, h * IB * A: (h + 1) * IB * A],
                                     w1_s,
                                     y1t[:, h * IB * A: (h + 1) * IB * A],
                                     start=True, stop=True)
                hg = hg_pool.tile([C, GW], bf16, tag="hg")
                if m * NG + q in RELU2_ACT:
                    nc.scalar.activation(hg[:], psh[:], Relu, bias=b1_s)
                else:
                    nc.vector.tensor_scalar(hg[:], psh[:], b1_s, 0.0,
                                            add_op, max_op)
                lo = m * PAIRS + q * GW
                nc.sync.dma_start(out[:, lo: lo + GW], hg[:])

            # software-pipelined emission, skew-1 between L1 and L2
            for idx in range(len(units) + 1):
                if idx < len(units):
                    emit_L1(idx)
                if idx >= 1:
                    emit_L2(idx - 1)
    nc.compile()
    return nc


def _get_compiled(fused=False):
    if fused not in _compiled:
        _compiled[fused] = _build(fused)
    return _compiled[fused]


def _shard_inputs(x, W0, b0, W1, b1):
    import ml_dtypes

    bf = ml_dtypes.bfloat16
    pf32 = np.stack([b0, b1], axis=1).astype(np.float32)  # [128, 2]
    w_cols = np.concatenate([W0[:F], W0[F:], W1], axis=1).astype(bf)
    in_maps = []
    for c in range(NCORES):
        xs = x[c * MPC: (c + 1) * MPC]                    # [MPC, A, F]
        xTs = xs.transpose(2, 0, 1).reshape(F, MPC * A)
        pb16 = np.ascontiguousarray(
            np.concatenate([xTs.astype(bf), w_cols], axis=1))
        in_maps.append({"pb16": pb16, "pf32": pf32})
    return in_maps


def _unshard(results):
    """[C, MPC*PAIRS] bf16 per core -> full (B, A*A, C) fp32 = H + H^T."""
    full = np.empty((B, A * A, C), dtype=np.float32)
    for c in range(NCORES):
        o = np.asarray(results[c]["out"], dtype=np.float32)
        for m in range(MPC):
            bidx = c * MPC + m
            h = o[:, m * PAIRS: (m + 1) * PAIRS].reshape(C, A, A)
            hsum = h + h.transpose(0, 2, 1)        # H[i,j] + H[j,i]
            full[bidx] = hsum.reshape(C, PAIRS).T
    return full


def kernel(x, W0, b0, W1, b1):
    from concourse.bass_utils import run_bass_kernel_spmd

    x = np.asarray(x, dtype=np.float32)
    W0 = np.asarray(W0, dtype=np.float32)
    b0 = np.asarray(b0, dtype=np.float32)
    W1 = np.asarray(W1, dtype=np.float32)
    b1 = np.asarray(b1, dtype=np.float32)

    in_maps = _shard_inputs(x, W0, b0, W1, b1)
    nc = _get_compiled(fused=False)
    res = run_bass_kernel_spmd(nc, in_maps, core_ids=list(range(NCORES)))
    return _unshard(res.results)
